# revision 11
# baseline (speedup 1.0000x reference)
"""Causal attention kernel for Trainium2 (8 NeuronCores, SPMD over heads).

Problem: B=4, H=16, S=2048, D=64, fp32.
  scores = Q @ K^T / sqrt(64); causal mask; softmax (global-max shift in the
  reference cancels exactly); out = attn @ V.

Distribution: B*H = 64 heads -> 8 heads per core, embarrassingly parallel.

Per-core algorithm (per head, two q-passes of 1024):
  - Q^T and K^T are duplicated into both partition halves so every matmul
    contracts over the full 128 partitions: uniform 128x128 PE tile mode
    (row-tiled 64-mode made LDWEIGHTS 2x slower and mode switches drain
    the PE). The duplicated contraction computes 2*(Q.K); the 2x is folded
    into the exp scale/coefficients.
  - exp is split across two engines running concurrently: ScalarE (exact
    exp, scale=1/8, plus a constant bias matching the DVE path's systematic
    relative bias) and the DVE via a custom 8-stage op:
    p = ((c0*z + c1)*z + c2)^16 == e^(z/8)*(1+eps), eps nearly constant,
    cancelling in the softmax ratio. Tile assignment balances the engines.
  - Causal diagonal block: GpSimd multiply by a triangular keep-mask.
  - PV: one 128-contraction matmul chain per k-tile into a single PSUM
    accumulator; [V|ones] gives the softmax denominator in row 64 for free.
  - Evacuation: ScalarE copies the low half, DVE the high half
    (concurrent); DMA out^T (+rowsum row) as [65, S] per head; the host
    does the final divide-by-rowsum and transpose.
"""

import math
import os
import sys

import numpy as np

if "/opt/trn_rl_repo" not in sys.path:
    sys.path.insert(0, "/opt/trn_rl_repo")

B, H, S, D = 4, 16, 2048, 64
N_CORES = 8
HEADS_PER_CORE = (B * H) // N_CORES  # 8
PASS_Q = 1024  # q-columns per pass (2 PSUM banks)
CHUNK = 512  # PSUM bank boundary for fp32 outputs

# DVE exp: p = (C0*z + C1)*z + C2, squared 4x, z the duplicated-contraction
# score 2*(Q.K) (exp arg z/16).  ScalarE path: exp(z*0.0625 + BETA).
# Jointly optimized so both paths agree through the softmax ratio.
EXP_C0 = 3.4436267949839664e-05 / 4.0
EXP_C1 = 7.770817159682695e-03 / 2.0
EXP_C2 = 0.9999542988018534
EXP_BETA = -8.692886851909931e-04

_EXP_OP = [None]


def _register_exp_op():
    if _EXP_OP[0] is not None:
        return _EXP_OP[0]
    import concourse.dve_ops as dve_ops
    from concourse.dve_ops import DveOp
    from concourse.dve_spec import C0, C1, C2, Spec, Src0, sq

    def _ref(in0, in1, s0, s1, imm2):
        p = ((in0.astype(np.float32) * s0 + s1) * in0 + imm2).astype(np.float32)
        for _ in range(4):
            p = (p * p).astype(np.float32)
        return p

    op = DveOp(
        "EXP_PK16_ANT",
        Spec(body=sq(sq(sq(sq((Src0 * C0 + C1) * Src0 + C2)))), reference=_ref),
        subdim=False,
        uops_sha={"v3": "b9028a2770b985b4", "v4": "8a0143ec7033f2f1"},
    )
    if op.name not in dve_ops._SUB_OPCODE_FOR_NAME:
        dve_ops.OPS.append(op)
        dve_ops._SUB_OPCODE_FOR_NAME[op.name] = max(
            dve_ops._SUB_OPCODE_FOR_NAME.values()
        ) + 1
        dve_ops.CUSTOM_DVE_SPECS[op.name] = op.spec
    _EXP_OP[0] = op
    return op


def _chunks(lo, hi):
    """Split [lo, hi) at absolute multiples of CHUNK (PSUM bank boundaries)."""
    out = []
    c = lo
    while c < hi:
        w = min(hi, (c // CHUNK + 1) * CHUNK) - c
        out.append((c, w))
        c += w
    return out


def build_attention(tc, outs, ins, n_heads=HEADS_PER_CORE, s=S, pass_q=PASS_Q):
    import concourse.bass as bass
    import concourse.mybir as mybir

    exp_op = _register_exp_op()

    nc = tc.nc
    f32 = mybir.dt.float32
    f16 = mybir.dt.float16
    Exp = mybir.ActivationFunctionType.Exp

    qt_d, kt_d, v_d = ins["qt"], ins["kt"], ins["v"]
    tri_d = ins["ctri"]
    ot_d = outs["ot"]

    n_ktiles = s // 128
    n_pass = s // pass_q
    ktiles_per_pass = pass_q // 128

    with (
        tc.tile_pool(name="consts", bufs=1) as cpool,
        tc.tile_pool(name="qpool", bufs=3) as qpool,
        tc.tile_pool(name="kpool", bufs=3) as kpool,
        tc.tile_pool(name="vpool", bufs=3) as vpool,
        tc.tile_pool(name="atpool", bufs=6) as atpool,
        tc.tile_pool(name="osbpool", bufs=2) as osbpool,
        tc.tile_pool(name="scpool", bufs=2, space="PSUM") as scpool,
        tc.tile_pool(name="accpool", bufs=2, space="PSUM") as accApool,
    ):
        c_tri = cpool.tile([128, 128], f16, tag="ctri")
        nc.sync.dma_start(c_tri[:], tri_d[:])
        c_beta = cpool.tile([128, 1], f32, tag="cbeta")
        nc.sync.dma_start(c_beta[:], ins["cbeta"][:])

        for h in range(n_heads):
            # Q^T duplicated into both partition halves (row-tile packing).
            qt2 = qpool.tile([128, s], f16)
            nc.sync.dma_start(qt2[0:64, :], qt_d[h])
            nc.sync.dma_start(qt2[64:128, :], qt_d[h])
            # K^T duplicated into both partition halves.
            kt2 = kpool.tile([128, s], f16)
            kt_src = kt_d[h].rearrange("d (t c) -> d t c", c=128)
            kt2_v = kt2.rearrange("p (t c) -> p t c", c=128)
            nc.sync.dma_start(kt2_v[0:64], kt_src)
            nc.sync.dma_start(kt2_v[64:128], kt_src)
            # V with a ones-column pre-appended on the host: [128, n_ktiles, 65].
            vx = vpool.tile([128, n_ktiles * 65], f16)
            vx_v = vx.rearrange("p (t c) -> p t c", c=65)
            nc.sync.dma_start(vx_v[:], v_d[h].rearrange("(t p) d -> p t d", p=128))

            for p in range(n_pass):
                q0 = p * pass_q
                kmax = (p + 1) * ktiles_per_pass
                acc = accApool.tile([65, pass_q], f32, name=f"acc_{h}_{p}", tag="acc")
                pv_queue = []

                def _emit_pv(entries):
                    for (k, at, qlo) in entries:
                        for (c, w) in _chunks(qlo - q0, pass_q):
                            co = c - (qlo - q0)
                            nc.tensor.matmul(
                                acc[0:65, c : c + w],
                                vx_v[:, k, :],
                                at[:, co : co + w],
                                start=(k == 0),
                                stop=(k == kmax - 1),
                                skip_group_check=True,
                            )

                # exp engine assignment: balance ScalarE (0.833ns/col+185)
                # vs DVE (1.04ns/col+125): DVE takes odd k-tiles except the
                # two largest odd spans per pass go to ScalarE.
                for kp in range(0, kmax, 2):
                    pair = [k for k in (kp, kp + 1) if k < kmax]
                    scs, spans, qlos = {}, {}, {}
                    for k in pair:
                        qlos[k] = max(q0, 128 * k)
                        spans[k] = q0 + pass_q - qlos[k]
                        scs[k] = scpool.tile(
                            [128, pass_q], f32, tag="sc", name=f"sc_{h}_{p}_{k}"
                        )
                    for k in pair:
                        for (c, w) in _chunks(0, spans[k]):
                            nc.tensor.matmul(
                                scs[k][:, c : c + w],
                                kt2_v[:, k],
                                qt2[:, qlos[k] + c : qlos[k] + c + w],
                                start=True,
                                stop=True,
                                skip_group_check=True,
                            )
                    cur = []
                    for k in pair:
                        span, qlo = spans[k], qlos[k]
                        at = atpool.tile([128, pass_q], f16)
                        if k % 2 == 0:
                            nc.scalar.activation(
                                at[:, 0:span], scs[k][:, 0:span], Exp,
                                bias=c_beta[:, 0:1], scale=0.0625,
                            )
                        else:
                            nc.vector._custom_dve(
                                exp_op,
                                out=at[:, 0:span],
                                in0=scs[k][:, 0:span],
                                s0=EXP_C0, s1=EXP_C1, imm2=EXP_C2,
                            )
                        if 128 * k >= q0:
                            # zero the masked upper part of the diagonal block
                            nc.gpsimd.tensor_mul(at[:, 0:128], at[:, 0:128], c_tri[:])
                        cur.append((k, at, qlo))
                    pv_queue.append(cur)
                    if len(pv_queue) > 2:
                        _emit_pv(pv_queue.pop(0))
                for entries in pv_queue:
                    _emit_pv(entries)
                # evacuate out^T (+rowsum row): half on ScalarE, half on
                # DVE (concurrent); DMA; host normalizes.
                osb = osbpool.tile([65, pass_q], f32, name=f"osb_{h}_{p}", tag="osb")
                nc.scalar.copy(osb[:, 0:512], acc[0:65, 0:512])
                nc.vector.tensor_copy(osb[:, 512:1024], acc[0:65, 512:1024])
                nc.sync.dma_start(ot_d[h, :, q0 : q0 + pass_q], osb[:])


def _make_consts():
    kk, qq = np.meshgrid(np.arange(128), np.arange(128), indexing="ij")
    tri = (kk <= qq).astype(np.float16)  # keep-mask for the diagonal block
    return tri


_NC_CACHE = {}


def _build_nc(n_heads=HEADS_PER_CORE, s=S, pass_q=PASS_Q):
    key = (n_heads, s, pass_q)
    if key in _NC_CACHE:
        return _NC_CACHE[key]
    import concourse.tile as tile
    from concourse import bacc, mybir

    nc = bacc.Bacc(
        "TRN2", target_bir_lowering=False, debug=False, enable_asserts=False
    )
    f32 = mybir.dt.float32
    f16 = mybir.dt.float16
    ins = {
        "qt": nc.dram_tensor("qt", [n_heads, D, s], f16, kind="ExternalInput").ap(),
        "kt": nc.dram_tensor("kt", [n_heads, D, s], f16, kind="ExternalInput").ap(),
        "v": nc.dram_tensor("v", [n_heads, s, D + 1], f16, kind="ExternalInput").ap(),
        "ctri": nc.dram_tensor("ctri", [128, 128], f16, kind="ExternalInput").ap(),
        "cbeta": nc.dram_tensor("cbeta", [128, 1], f32, kind="ExternalInput").ap(),
    }
    outs = {
        "ot": nc.dram_tensor("ot", [n_heads, 65, s], f32, kind="ExternalOutput").ap(),
    }
    with tile.TileContext(nc) as tc:
        build_attention(tc, outs, ins, n_heads=n_heads, s=s, pass_q=pass_q)
    nc.compile()
    _NC_CACHE[key] = nc
    return nc


def kernel(Q, K, V, mask, trace=False):
    """Full-input entry point: shards over 8 NeuronCores, returns full output."""
    from concourse.bass_utils import run_bass_kernel_spmd

    nc = _build_nc()
    tri = _make_consts()

    Qf = np.ascontiguousarray(
        Q.reshape(B * H, S, D).transpose(0, 2, 1), dtype=np.float16
    )
    Kf = np.ascontiguousarray(
        K.reshape(B * H, S, D).transpose(0, 2, 1), dtype=np.float16
    )
    Vf = np.concatenate(
        [
            V.reshape(B * H, S, D).astype(np.float16),
            np.ones((B * H, S, 1), dtype=np.float16),
        ],
        axis=-1,
    )

    in_maps = []
    for c in range(N_CORES):
        sl = slice(c * HEADS_PER_CORE, (c + 1) * HEADS_PER_CORE)
        in_maps.append(
            {
                "qt": Qf[sl],
                "kt": Kf[sl],
                "v": Vf[sl],
                "ctri": tri,
                "cbeta": np.full((128, 1), EXP_BETA, dtype=np.float32),
            }
        )

    res = run_bass_kernel_spmd(nc, in_maps, core_ids=list(range(N_CORES)), trace=trace)
    ot = np.concatenate([res.results[c]["ot"] for c in range(N_CORES)], axis=0)
    # ot: [B*H, 65, S] -- rows 0..63 are out^T columns, row 64 the rowsum.
    out = (ot[:, :64, :] / ot[:, 64:65, :]).transpose(0, 2, 1)
    out = out.reshape(B, H, S, D)
    kernel.last_results = res
    return np.ascontiguousarray(out, dtype=np.float32)


# revision 12
# speedup vs baseline: 1.3142x; 1.3142x over previous
"""Causal attention kernel for Trainium2 (8 NeuronCores, SPMD over heads).

Problem: B=4, H=16, S=2048, D=64, fp32.
  scores = Q @ K^T / sqrt(64); causal mask; softmax (global-max shift in the
  reference cancels exactly); out = attn @ V.

Distribution: B*H = 64 heads -> 8 heads per core, embarrassingly parallel.

Per-core algorithm (per head, two q-passes of 1024):
  - Q^T and K^T are duplicated into both partition halves so every matmul
    contracts over the full 128 partitions: uniform 128x128 PE tile mode
    (row-tiled 64-mode made LDWEIGHTS 2x slower and mode switches drain
    the PE). The duplicated contraction computes 2*(Q.K); the 2x is folded
    into the exp scale/coefficients.
  - exp is split across two engines running concurrently: ScalarE (exact
    exp, scale=1/8, plus a constant bias matching the DVE path's systematic
    relative bias) and the DVE via a custom 8-stage op:
    p = ((c0*z + c1)*z + c2)^16 == e^(z/8)*(1+eps), eps nearly constant,
    cancelling in the softmax ratio. Tile assignment balances the engines.
  - Causal diagonal block: GpSimd multiply by a triangular keep-mask.
  - PV: one 128-contraction matmul chain per k-tile into a single PSUM
    accumulator; [V|ones] gives the softmax denominator in row 64 for free.
  - Evacuation: ScalarE copies the low half, DVE the high half
    (concurrent); DMA out^T (+rowsum row) as [65, S] per head; the host
    does the final divide-by-rowsum and transpose.
"""

import math
import os
import sys

import numpy as np

if "/opt/trn_rl_repo" not in sys.path:
    sys.path.insert(0, "/opt/trn_rl_repo")

B, H, S, D = 4, 16, 2048, 64
N_CORES = 8
HEADS_PER_CORE = (B * H) // N_CORES  # 8
PASS_Q = 1024  # q-columns per pass (2 PSUM banks)
CHUNK = 512  # PSUM bank boundary for fp32 outputs

# DVE exp: p = (C0*z + C1)*z + C2, squared 4x, z the duplicated-contraction
# score 2*(Q.K) (exp arg z/16).  ScalarE path: exp(z*0.0625 + BETA).
# Jointly optimized so both paths agree through the softmax ratio.
EXP_C0 = 3.4436267949839664e-05 / 4.0
EXP_C1 = 7.770817159682695e-03 / 2.0
EXP_C2 = 0.9999542988018534
EXP_BETA = -8.692886851909931e-04

_EXP_OP = [None]


def _register_exp_op():
    if _EXP_OP[0] is not None:
        return _EXP_OP[0]
    import concourse.dve_ops as dve_ops
    from concourse.dve_ops import DveOp
    from concourse.dve_spec import C0, C1, C2, Spec, Src0, sq

    def _ref(in0, in1, s0, s1, imm2):
        p = ((in0.astype(np.float32) * s0 + s1) * in0 + imm2).astype(np.float32)
        for _ in range(4):
            p = (p * p).astype(np.float32)
        return p

    op = DveOp(
        "EXP_PK16_ANT",
        Spec(body=sq(sq(sq(sq((Src0 * C0 + C1) * Src0 + C2)))), reference=_ref),
        subdim=False,
        uops_sha={"v3": "b9028a2770b985b4", "v4": "8a0143ec7033f2f1"},
    )
    if op.name not in dve_ops._SUB_OPCODE_FOR_NAME:
        dve_ops.OPS.append(op)
        dve_ops._SUB_OPCODE_FOR_NAME[op.name] = max(
            dve_ops._SUB_OPCODE_FOR_NAME.values()
        ) + 1
        dve_ops.CUSTOM_DVE_SPECS[op.name] = op.spec
    _EXP_OP[0] = op
    return op


def _chunks(lo, hi):
    """Split [lo, hi) at absolute multiples of CHUNK (PSUM bank boundaries)."""
    out = []
    c = lo
    while c < hi:
        w = min(hi, (c // CHUNK + 1) * CHUNK) - c
        out.append((c, w))
        c += w
    return out


def build_attention(tc, outs, ins, n_heads=HEADS_PER_CORE, s=S, pass_q=PASS_Q):
    import concourse.bass as bass
    import concourse.mybir as mybir

    exp_op = _register_exp_op()

    nc = tc.nc
    f32 = mybir.dt.float32
    f16 = mybir.dt.float16
    Exp = mybir.ActivationFunctionType.Exp

    qt_d, kt_d, v_d = ins["qt"], ins["kt"], ins["v"]
    tri_d = ins["ctri"]
    ot_d = outs["ot"]

    n_ktiles = s // 128
    n_pass = s // pass_q
    ktiles_per_pass = pass_q // 128

    with (
        tc.tile_pool(name="consts", bufs=1) as cpool,
        tc.tile_pool(name="qpool", bufs=3) as qpool,
        tc.tile_pool(name="kpool", bufs=3) as kpool,
        tc.tile_pool(name="vpool", bufs=3) as vpool,
        tc.tile_pool(name="atpool", bufs=8) as atpool,
        tc.tile_pool(name="osbpool", bufs=2) as osbpool,
        tc.tile_pool(name="scpool", bufs=3, space="PSUM") as scpool,
        tc.tile_pool(name="accpool", bufs=1, space="PSUM") as accApool,
    ):
        c_tri = cpool.tile([128, 128], f16, tag="ctri")
        nc.sync.dma_start(c_tri[:], tri_d[:])
        c_beta = cpool.tile([128, 1], f32, tag="cbeta")
        nc.sync.dma_start(c_beta[:], ins["cbeta"][:])

        for h in range(n_heads):
            # Q^T duplicated into both partition halves (row-tile packing).
            qt2 = qpool.tile([128, s], f16)
            nc.sync.dma_start(qt2[0:64, :], qt_d[h])
            nc.sync.dma_start(qt2[64:128, :], qt_d[h])
            # K^T duplicated into both partition halves.
            kt2 = kpool.tile([128, s], f16)
            kt_src = kt_d[h].rearrange("d (t c) -> d t c", c=128)
            kt2_v = kt2.rearrange("p (t c) -> p t c", c=128)
            nc.sync.dma_start(kt2_v[0:64], kt_src)
            nc.sync.dma_start(kt2_v[64:128], kt_src)
            # V with a ones-column pre-appended on the host: [128, n_ktiles, 65].
            vx = vpool.tile([128, n_ktiles * 65], f16)
            vx_v = vx.rearrange("p (t c) -> p t c", c=65)
            nc.sync.dma_start(vx_v[:], v_d[h].rearrange("(t p) d -> p t d", p=128))

            for p in range(n_pass):
                q0 = p * pass_q
                kmax = (p + 1) * ktiles_per_pass
                acc = accApool.tile([65, pass_q], f32, name=f"acc_{h}_{p}", tag="acc")
                pv_queue = []

                def _emit_pv(entries):
                    for (k, at, qlo) in entries:
                        for (c, w) in _chunks(qlo - q0, pass_q):
                            co = c - (qlo - q0)
                            nc.tensor.matmul(
                                acc[0:65, c : c + w],
                                vx_v[:, k, :],
                                at[:, co : co + w],
                                start=(k == 0),
                                stop=(k == kmax - 1),
                                skip_group_check=True,
                            )

                # exp engine assignment: balance ScalarE (0.833ns/col+185)
                # vs DVE (1.04ns/col+125): DVE takes odd k-tiles except the
                # two largest odd spans per pass go to ScalarE.
                for kp in range(0, kmax, 2):
                    pair = [k for k in (kp, kp + 1) if k < kmax]
                    scs, spans, qlos = {}, {}, {}
                    for k in pair:
                        qlos[k] = max(q0, 128 * k)
                        spans[k] = q0 + pass_q - qlos[k]
                        scs[k] = scpool.tile(
                            [128, pass_q], f32, tag="sc", name=f"sc_{h}_{p}_{k}"
                        )
                    for k in pair:
                        for (c, w) in _chunks(0, spans[k]):
                            nc.tensor.matmul(
                                scs[k][:, c : c + w],
                                kt2_v[:, k],
                                qt2[:, qlos[k] + c : qlos[k] + c + w],
                                start=True,
                                stop=True,
                                skip_group_check=True,
                            )
                    cur = []
                    for k in pair:
                        span, qlo = spans[k], qlos[k]
                        at = atpool.tile([128, pass_q], f16)
                        if k % 2 == 0:
                            nc.scalar.activation(
                                at[:, 0:span], scs[k][:, 0:span], Exp,
                                bias=c_beta[:, 0:1], scale=0.0625,
                            )
                        else:
                            nc.vector._custom_dve(
                                exp_op,
                                out=at[:, 0:span],
                                in0=scs[k][:, 0:span],
                                s0=EXP_C0, s1=EXP_C1, imm2=EXP_C2,
                            )
                        if 128 * k >= q0:
                            # zero the masked upper part of the diagonal block
                            nc.gpsimd.tensor_mul(at[:, 0:128], at[:, 0:128], c_tri[:])
                        cur.append((k, at, qlo))
                    pv_queue.append(cur)
                    if len(pv_queue) > 2:
                        _emit_pv(pv_queue.pop(0))
                for entries in pv_queue:
                    _emit_pv(entries)
                # evacuate out^T (+rowsum row): half on ScalarE, half on
                # DVE (concurrent); DMA; host normalizes.
                osb = osbpool.tile([65, pass_q], f32, name=f"osb_{h}_{p}", tag="osb")
                nc.scalar.copy(osb[:, 0:512], acc[0:65, 0:512])
                nc.vector.tensor_copy(osb[:, 512:1024], acc[0:65, 512:1024])
                nc.sync.dma_start(ot_d[h, :, q0 : q0 + pass_q], osb[:])


def _make_consts():
    kk, qq = np.meshgrid(np.arange(128), np.arange(128), indexing="ij")
    tri = (kk <= qq).astype(np.float16)  # keep-mask for the diagonal block
    return tri


_NC_CACHE = {}


def _build_nc(n_heads=HEADS_PER_CORE, s=S, pass_q=PASS_Q):
    key = (n_heads, s, pass_q)
    if key in _NC_CACHE:
        return _NC_CACHE[key]
    import concourse.tile as tile
    from concourse import bacc, mybir

    nc = bacc.Bacc(
        "TRN2", target_bir_lowering=False, debug=False, enable_asserts=False
    )
    f32 = mybir.dt.float32
    f16 = mybir.dt.float16
    ins = {
        "qt": nc.dram_tensor("qt", [n_heads, D, s], f16, kind="ExternalInput").ap(),
        "kt": nc.dram_tensor("kt", [n_heads, D, s], f16, kind="ExternalInput").ap(),
        "v": nc.dram_tensor("v", [n_heads, s, D + 1], f16, kind="ExternalInput").ap(),
        "ctri": nc.dram_tensor("ctri", [128, 128], f16, kind="ExternalInput").ap(),
        "cbeta": nc.dram_tensor("cbeta", [128, 1], f32, kind="ExternalInput").ap(),
    }
    outs = {
        "ot": nc.dram_tensor("ot", [n_heads, 65, s], f32, kind="ExternalOutput").ap(),
    }
    with tile.TileContext(nc) as tc:
        build_attention(tc, outs, ins, n_heads=n_heads, s=s, pass_q=pass_q)
    nc.compile()
    _NC_CACHE[key] = nc
    return nc


def kernel(Q, K, V, mask, trace=False):
    """Full-input entry point: shards over 8 NeuronCores, returns full output."""
    from concourse.bass_utils import run_bass_kernel_spmd

    nc = _build_nc()
    tri = _make_consts()

    Qf = np.ascontiguousarray(
        Q.reshape(B * H, S, D).transpose(0, 2, 1), dtype=np.float16
    )
    Kf = np.ascontiguousarray(
        K.reshape(B * H, S, D).transpose(0, 2, 1), dtype=np.float16
    )
    Vf = np.concatenate(
        [
            V.reshape(B * H, S, D).astype(np.float16),
            np.ones((B * H, S, 1), dtype=np.float16),
        ],
        axis=-1,
    )

    in_maps = []
    for c in range(N_CORES):
        sl = slice(c * HEADS_PER_CORE, (c + 1) * HEADS_PER_CORE)
        in_maps.append(
            {
                "qt": Qf[sl],
                "kt": Kf[sl],
                "v": Vf[sl],
                "ctri": tri,
                "cbeta": np.full((128, 1), EXP_BETA, dtype=np.float32),
            }
        )

    res = run_bass_kernel_spmd(nc, in_maps, core_ids=list(range(N_CORES)), trace=trace)
    ot = np.concatenate([res.results[c]["ot"] for c in range(N_CORES)], axis=0)
    # ot: [B*H, 65, S] -- rows 0..63 are out^T columns, row 64 the rowsum.
    out = (ot[:, :64, :] / ot[:, 64:65, :]).transpose(0, 2, 1)
    out = out.reshape(B, H, S, D)
    kernel.last_results = res
    return np.ascontiguousarray(out, dtype=np.float32)


# revision 13
# speedup vs baseline: 1.3558x; 1.0317x over previous
"""Causal attention kernel for Trainium2 (8 NeuronCores, SPMD over heads).

Problem: B=4, H=16, S=2048, D=64, fp32.
  scores = Q @ K^T / sqrt(64); causal mask; softmax (global-max shift in the
  reference cancels exactly); out = attn @ V.

Distribution: B*H = 64 heads -> 8 heads per core, embarrassingly parallel.

Per-core algorithm (per head, two q-passes of 1024):
  - Q^T and K^T are duplicated into both partition halves so every matmul
    contracts over the full 128 partitions: uniform 128x128 PE tile mode
    (row-tiled 64-mode made LDWEIGHTS 2x slower and mode switches drain
    the PE). The duplicated contraction computes 2*(Q.K); the 2x is folded
    into the exp scale/coefficients.
  - exp is split across two engines running concurrently: ScalarE (exact
    exp, scale=1/8, plus a constant bias matching the DVE path's systematic
    relative bias) and the DVE via a custom 8-stage op:
    p = ((c0*z + c1)*z + c2)^16 == e^(z/8)*(1+eps), eps nearly constant,
    cancelling in the softmax ratio. Tile assignment balances the engines.
  - Causal diagonal block: GpSimd multiply by a triangular keep-mask.
  - PV: one 128-contraction matmul chain per k-tile into a single PSUM
    accumulator; [V|ones] gives the softmax denominator in row 64 for free.
  - Evacuation: ScalarE copies the low half, DVE the high half
    (concurrent); DMA out^T (+rowsum row) as [65, S] per head; the host
    does the final divide-by-rowsum and transpose.
"""

import math
import os
import sys

import numpy as np

if "/opt/trn_rl_repo" not in sys.path:
    sys.path.insert(0, "/opt/trn_rl_repo")

B, H, S, D = 4, 16, 2048, 64
N_CORES = 8
HEADS_PER_CORE = (B * H) // N_CORES  # 8
PASS_Q = 1024  # q-columns per pass (2 PSUM banks)
CHUNK = 512  # PSUM bank boundary for fp32 outputs

# DVE exp: p = (C0*z + C1)*z + C2, squared 4x, z the duplicated-contraction
# score 2*(Q.K) (exp arg z/16).  ScalarE path: exp(z*0.0625 + BETA).
# Jointly optimized so both paths agree through the softmax ratio.
EXP_C0 = 3.4436267949839664e-05 / 4.0
EXP_C1 = 7.770817159682695e-03 / 2.0
EXP_C2 = 0.9999542988018534
EXP_BETA = -8.692886851909931e-04

_EXP_OP = [None]


def _register_exp_op():
    if _EXP_OP[0] is not None:
        return _EXP_OP[0]
    import concourse.dve_ops as dve_ops
    from concourse.dve_ops import DveOp
    from concourse.dve_spec import C0, C1, C2, Spec, Src0, sq

    def _ref(in0, in1, s0, s1, imm2):
        p = ((in0.astype(np.float32) * s0 + s1) * in0 + imm2).astype(np.float32)
        for _ in range(4):
            p = (p * p).astype(np.float32)
        return p

    op = DveOp(
        "EXP_PK16_ANT",
        Spec(body=sq(sq(sq(sq((Src0 * C0 + C1) * Src0 + C2)))), reference=_ref),
        subdim=False,
        uops_sha={"v3": "b9028a2770b985b4", "v4": "8a0143ec7033f2f1"},
    )
    if op.name not in dve_ops._SUB_OPCODE_FOR_NAME:
        dve_ops.OPS.append(op)
        dve_ops._SUB_OPCODE_FOR_NAME[op.name] = max(
            dve_ops._SUB_OPCODE_FOR_NAME.values()
        ) + 1
        dve_ops.CUSTOM_DVE_SPECS[op.name] = op.spec
    _EXP_OP[0] = op
    return op


def _chunks(lo, hi):
    """Split [lo, hi) at absolute multiples of CHUNK (PSUM bank boundaries)."""
    out = []
    c = lo
    while c < hi:
        w = min(hi, (c // CHUNK + 1) * CHUNK) - c
        out.append((c, w))
        c += w
    return out


def build_attention(tc, outs, ins, n_heads=HEADS_PER_CORE, s=S, pass_q=PASS_Q):
    import concourse.bass as bass
    import concourse.mybir as mybir

    exp_op = _register_exp_op()

    nc = tc.nc
    f32 = mybir.dt.float32
    f16 = mybir.dt.float16
    Exp = mybir.ActivationFunctionType.Exp

    qt_d, kt_d, v_d = ins["qt"], ins["kt"], ins["v"]
    tri_d = ins["ctri"]
    ot_d = outs["ot"]

    n_ktiles = s // 128
    n_pass = s // pass_q
    ktiles_per_pass = pass_q // 128

    with (
        tc.tile_pool(name="consts", bufs=1) as cpool,
        tc.tile_pool(name="qpool", bufs=3) as qpool,
        tc.tile_pool(name="kpool", bufs=3) as kpool,
        tc.tile_pool(name="vpool", bufs=3) as vpool,
        tc.tile_pool(name="atpool", bufs=8) as atpool,
        tc.tile_pool(name="osbpool", bufs=2) as osbpool,
        tc.tile_pool(name="scpool", bufs=3, space="PSUM") as scpool,
        tc.tile_pool(name="accpool", bufs=1, space="PSUM") as accApool,
    ):
        c_tri = cpool.tile([128, 128], f16, tag="ctri")
        nc.sync.dma_start(c_tri[:], tri_d[:])
        c_beta = cpool.tile([128, 1], f32, tag="cbeta")
        nc.sync.dma_start(c_beta[:], ins["cbeta"][:])
        # dummy exp: pulls ACT_TABLE_LOAD into the startup DMA window
        warm = cpool.tile([128, 1], f16, tag="warm")
        nc.scalar.activation(warm[:], c_beta[:], Exp, scale=0.0)

        for h in range(n_heads):
            # Q^T/K^T duplicated into both partition halves; all loads are
            # split at the pass boundary so pass 0 can start after half the
            # bytes land.
            hs = s // 2
            qt2 = qpool.tile([128, s], f16)
            kt2 = kpool.tile([128, s], f16)
            kt_src = kt_d[h].rearrange("d (t c) -> d t c", c=128)
            kt2_v = kt2.rearrange("p (t c) -> p t c", c=128)
            vx = vpool.tile([128, n_ktiles * 65], f16)
            vx_v = vx.rearrange("p (t c) -> p t c", c=65)
            v_src = v_d[h].rearrange("(t p) d -> p t d", p=128)
            ht = n_ktiles // 2
            nc.sync.dma_start(kt2_v[0:64, 0:ht], kt_src[:, 0:ht])
            nc.sync.dma_start(kt2_v[64:128, 0:ht], kt_src[:, 0:ht])
            nc.sync.dma_start(qt2[0:64, 0:hs], qt_d[h, :, 0:hs])
            nc.sync.dma_start(qt2[64:128, 0:hs], qt_d[h, :, 0:hs])
            nc.sync.dma_start(vx_v[:, 0:ht], v_src[:, 0:ht])
            nc.sync.dma_start(kt2_v[0:64, ht:], kt_src[:, ht:])
            nc.sync.dma_start(kt2_v[64:128, ht:], kt_src[:, ht:])
            nc.sync.dma_start(qt2[0:64, hs:], qt_d[h, :, hs:])
            nc.sync.dma_start(qt2[64:128, hs:], qt_d[h, :, hs:])
            nc.sync.dma_start(vx_v[:, ht:], v_src[:, ht:])

            for p in range(n_pass):
                q0 = p * pass_q
                kmax = (p + 1) * ktiles_per_pass
                acc = accApool.tile([65, pass_q], f32, name=f"acc_{h}_{p}", tag="acc")
                pv_queue = []

                def _emit_pv(entries):
                    for (k, at, qlo) in entries:
                        for (c, w) in _chunks(qlo - q0, pass_q):
                            co = c - (qlo - q0)
                            nc.tensor.matmul(
                                acc[0:65, c : c + w],
                                vx_v[:, k, :],
                                at[:, co : co + w],
                                start=(k == 0),
                                stop=(k == kmax - 1),
                                skip_group_check=True,
                            )

                # exp engine assignment: balance ScalarE (0.833ns/col+185)
                # vs DVE (1.04ns/col+125): DVE takes odd k-tiles except the
                # two largest odd spans per pass go to ScalarE.
                for kp in range(0, kmax, 2):
                    pair = [k for k in (kp, kp + 1) if k < kmax]
                    scs, spans, qlos = {}, {}, {}
                    for k in pair:
                        qlos[k] = max(q0, 128 * k)
                        spans[k] = q0 + pass_q - qlos[k]
                        scs[k] = scpool.tile(
                            [128, pass_q], f32, tag="sc", name=f"sc_{h}_{p}_{k}"
                        )
                    for k in pair:
                        for (c, w) in _chunks(0, spans[k]):
                            nc.tensor.matmul(
                                scs[k][:, c : c + w],
                                kt2_v[:, k],
                                qt2[:, qlos[k] + c : qlos[k] + c + w],
                                start=True,
                                stop=True,
                                skip_group_check=True,
                            )
                    cur = []
                    for k in pair:
                        span, qlo = spans[k], qlos[k]
                        at = atpool.tile([128, pass_q], f16)
                        if k % 2 == 0:
                            nc.scalar.activation(
                                at[:, 0:span], scs[k][:, 0:span], Exp,
                                bias=c_beta[:, 0:1], scale=0.0625,
                            )
                        else:
                            nc.vector._custom_dve(
                                exp_op,
                                out=at[:, 0:span],
                                in0=scs[k][:, 0:span],
                                s0=EXP_C0, s1=EXP_C1, imm2=EXP_C2,
                            )
                        if 128 * k >= q0:
                            # zero the masked upper part of the diagonal block
                            nc.gpsimd.tensor_mul(at[:, 0:128], at[:, 0:128], c_tri[:])
                        cur.append((k, at, qlo))
                    pv_queue.append(cur)
                    if len(pv_queue) > 2:
                        _emit_pv(pv_queue.pop(0))
                for entries in pv_queue:
                    _emit_pv(entries)
                # evacuate out^T (+rowsum row): half on ScalarE, half on
                # DVE (concurrent), each half DMA'd as soon as it lands.
                osb = osbpool.tile([65, pass_q], f32, name=f"osb_{h}_{p}", tag="osb")
                nc.scalar.copy(osb[:, 0:512], acc[0:65, 0:512])
                nc.sync.dma_start(ot_d[h, :, q0 : q0 + 512], osb[:, 0:512])
                nc.vector.tensor_copy(osb[:, 512:1024], acc[0:65, 512:1024])
                nc.sync.dma_start(ot_d[h, :, q0 + 512 : q0 + 1024], osb[:, 512:1024])


def _make_consts():
    kk, qq = np.meshgrid(np.arange(128), np.arange(128), indexing="ij")
    tri = (kk <= qq).astype(np.float16)  # keep-mask for the diagonal block
    return tri


_NC_CACHE = {}


def _build_nc(n_heads=HEADS_PER_CORE, s=S, pass_q=PASS_Q):
    key = (n_heads, s, pass_q)
    if key in _NC_CACHE:
        return _NC_CACHE[key]
    import concourse.tile as tile
    from concourse import bacc, mybir

    nc = bacc.Bacc(
        "TRN2", target_bir_lowering=False, debug=False, enable_asserts=False
    )
    f32 = mybir.dt.float32
    f16 = mybir.dt.float16
    ins = {
        "qt": nc.dram_tensor("qt", [n_heads, D, s], f16, kind="ExternalInput").ap(),
        "kt": nc.dram_tensor("kt", [n_heads, D, s], f16, kind="ExternalInput").ap(),
        "v": nc.dram_tensor("v", [n_heads, s, D + 1], f16, kind="ExternalInput").ap(),
        "ctri": nc.dram_tensor("ctri", [128, 128], f16, kind="ExternalInput").ap(),
        "cbeta": nc.dram_tensor("cbeta", [128, 1], f32, kind="ExternalInput").ap(),
    }
    outs = {
        "ot": nc.dram_tensor("ot", [n_heads, 65, s], f32, kind="ExternalOutput").ap(),
    }
    with tile.TileContext(nc) as tc:
        build_attention(tc, outs, ins, n_heads=n_heads, s=s, pass_q=pass_q)
    nc.compile()
    _NC_CACHE[key] = nc
    return nc


def kernel(Q, K, V, mask, trace=False):
    """Full-input entry point: shards over 8 NeuronCores, returns full output."""
    from concourse.bass_utils import run_bass_kernel_spmd

    nc = _build_nc()
    tri = _make_consts()

    Qf = np.ascontiguousarray(
        Q.reshape(B * H, S, D).transpose(0, 2, 1), dtype=np.float16
    )
    Kf = np.ascontiguousarray(
        K.reshape(B * H, S, D).transpose(0, 2, 1), dtype=np.float16
    )
    Vf = np.concatenate(
        [
            V.reshape(B * H, S, D).astype(np.float16),
            np.ones((B * H, S, 1), dtype=np.float16),
        ],
        axis=-1,
    )

    in_maps = []
    for c in range(N_CORES):
        sl = slice(c * HEADS_PER_CORE, (c + 1) * HEADS_PER_CORE)
        in_maps.append(
            {
                "qt": Qf[sl],
                "kt": Kf[sl],
                "v": Vf[sl],
                "ctri": tri,
                "cbeta": np.full((128, 1), EXP_BETA, dtype=np.float32),
            }
        )

    res = run_bass_kernel_spmd(nc, in_maps, core_ids=list(range(N_CORES)), trace=trace)
    ot = np.concatenate([res.results[c]["ot"] for c in range(N_CORES)], axis=0)
    # ot: [B*H, 65, S] -- rows 0..63 are out^T columns, row 64 the rowsum.
    out = (ot[:, :64, :] / ot[:, 64:65, :]).transpose(0, 2, 1)
    out = out.reshape(B, H, S, D)
    kernel.last_results = res
    return np.ascontiguousarray(out, dtype=np.float32)
